# revision 1
# baseline (speedup 1.0000x reference)
"""Single-head causal self-attention on 8 Trainium2 NeuronCores.

Problem: x[8, 2048, 1024], Wq/Wk/Wv[1024, 64] ->
  out[b] = softmax(causal((x[b]@Wq) @ (x[b]@Wk)^T / 8)) @ (x[b]@Wv)

Sharding: data-parallel over batch B=8, one batch element per core; weights
replicated. x is transposed host-side per core and Wq|Wk are concatenated so
every on-device matmul contracts over the SBUF partition dim with dense DMAs.

Per-core scheme ("transposed scores"):
  - [q^T;k^T] = Wqk^T @ x^T   (PE, fused, evacuated into two base-0 tiles via
    partition-shifted ACT copies)
  - v^T = Wv^T @ x^T, then PE-transpose -> V[2048, 64] (+ ones column)
  - S^T[j-tile, q-chunk] = (k^T tile)^T @ q^T, causal blocks only
  - P^T = exp(S^T / 8)  (ACT, PSUM->SBUF; no max-subtraction: scores ~N(0,1))
  - diagonal blocks: multiply boundary 128-col sub-block by a 0/1 triangle
    mask; columns left of it are skipped entirely (matmuls operate on slices)
  - out^T[qc] = sum_j V_aug[j]^T @ P^T ; ones column makes row 64 the softmax
    denominator for free
  - PE-transpose out^T -> [q, 65]; multiply rows by reciprocal of col 64 (DVE)
"""

import numpy as np

import concourse.bass as bass
import concourse.mybir as mybir
import concourse.tile as tile
from concourse import bacc
from concourse.bass_utils import run_bass_kernel_spmd
from concourse.masks import make_identity, make_upper_triangular

N_CORES = 8
B, T, C, D = 8, 2048, 1024, 64
CT = C // 128          # 8 contraction tiles
NT = T // 128          # 16 row tiles
QCHUNK = 512
NQC = T // QCHUNK      # 4 q-chunks
JPER = QCHUNK // 128   # 4 j-tiles per q-chunk
SCALE = float(1.0 / np.sqrt(D))

FP = mybir.dt.float32
MM_DT = mybir.dt.float32r  # matmul ingest dtype; FP = exact but 4x slower


def build_nc():
    nc = bacc.Bacc("TRN2", target_bir_lowering=False)
    xT_h = nc.dram_tensor("xT", [C, T], MM_DT, kind="ExternalInput")
    wqk_h = nc.dram_tensor("wqk", [C, 128], MM_DT, kind="ExternalInput")
    wv_h = nc.dram_tensor("wv", [C, D], MM_DT, kind="ExternalInput")
    y_h = nc.dram_tensor("y", [T, D], FP, kind="ExternalOutput")

    with tile.TileContext(nc) as tc:
        with (
            tc.tile_pool(name="const", bufs=1) as const,
            tc.tile_pool(name="pt", bufs=4) as ptp,
            tc.tile_pool(name="otp", bufs=2) as otp,
            tc.tile_pool(name="ps_s", bufs=3, space="PSUM") as ps_s,
            tc.tile_pool(name="ps_p", bufs=1, space="PSUM") as ps_p,
            tc.tile_pool(name="ps_o", bufs=2, space="PSUM") as ps_o,
            tc.tile_pool(name="ps_t", bufs=1, space="PSUM") as ps_t,
        ):
            # ---- constants ----
            ident = const.tile([128, 128], FP, tag="ident")
            make_identity(nc, ident)
            tri = const.tile([128, 128], FP, tag="tri")  # tri[p,f]=1.0 iff f>=p
            make_upper_triangular(nc, tri, val=1.0, diag=True)

            wqk_sb = const.tile([128, CT, 128], MM_DT, tag="wqk")
            nc.sync.dma_start(
                out=wqk_sb, in_=wqk_h[:, :].rearrange("(ct p) m -> p ct m", p=128)
            )
            wv_sb = const.tile([128, CT, D], MM_DT, tag="wv")
            nc.sync.dma_start(
                out=wv_sb, in_=wv_h[:, :].rearrange("(ct p) m -> p ct m", p=128)
            )

            xT_sb = const.tile([128, CT, T], MM_DT, tag="xT")
            qT = const.tile([64, T], MM_DT, tag="qT")
            kT = const.tile([64, T], MM_DT, tag="kT")
            vT = const.tile([64, T], FP, tag="vT")
            V = const.tile([128, NT, D + 1], MM_DT, tag="V")  # col D = ones
            ones_col = const.tile([128, NT], FP, tag="ones")
            nc.gpsimd.memset(ones_col, 1.0)
            nc.scalar.copy(V[:, :, D], ones_col)
            out_sb = const.tile([128, NT, D], FP, tag="out")

            xT_in = xT_h[:, :].rearrange("(ct p) t -> p ct t", p=128)

            # ---- DMA + projections, pipelined per t-chunk ----
            for tcu in range(NQC):
                sl = slice(tcu * QCHUNK, (tcu + 1) * QCHUNK)
                nc.sync.dma_start(out=xT_sb[:, :, sl], in_=xT_in[:, :, sl])

                p_qk = ps_p.tile([128, QCHUNK], FP, tag="qk")
                for ct in range(CT):
                    nc.tensor.matmul(
                        p_qk,
                        wqk_sb[:, ct, :],
                        xT_sb[:, ct, sl],
                        start=(ct == 0),
                        stop=(ct == CT - 1),
                    )
                nc.scalar.copy(qT[:, sl], p_qk[0:64, :])
                nc.scalar.copy(kT[:, sl], p_qk[64:128, :])  # partition shift

                p_v = ps_p.tile([64, QCHUNK], FP, tag="v")
                for ct in range(CT):
                    nc.tensor.matmul(
                        p_v,
                        wv_sb[:, ct, :],
                        xT_sb[:, ct, sl],
                        start=(ct == 0),
                        stop=(ct == CT - 1),
                    )
                nc.scalar.copy(vT[:, sl], p_v)

                # V natural layout for the j-tiles of this chunk
                for i in range(JPER):
                    jt = tcu * JPER + i
                    p_vt = ps_t.tile([128, D + 1], FP, tag="t")
                    nc.tensor.transpose(
                        p_vt[:, 0:D],
                        vT[:, jt * 128 : (jt + 1) * 128],
                        ident[0:64, 0:64],
                    )
                    nc.scalar.copy(V[:, jt, 0:D], p_vt[:, 0:D])

                # ---- attention for q-chunk qc = tcu (needs k/v chunks <= tcu) ----
                qc = tcu
                p_out = ps_o.tile([D + 1, QCHUNK], FP, tag="o")
                n_jt = qc * JPER + JPER
                blocks = []
                for jt in range(n_jt):
                    i = jt - qc * JPER  # >=0 on diagonal j-tiles
                    lo = max(i, 0) * 128  # first valid column of this block
                    blocks.append((jt, lo))

                def s_block(jt, lo):
                    p_s = ps_s.tile([128, QCHUNK], FP, tag="s")
                    nc.tensor.matmul(
                        p_s[:, lo:QCHUNK],
                        kT[:, jt * 128 : (jt + 1) * 128],
                        qT[:, qc * QCHUNK + lo : (qc + 1) * QCHUNK],
                        start=True,
                        stop=True,
                    )
                    pt = ptp.tile([128, QCHUNK], MM_DT, tag="pt")
                    nc.scalar.activation(
                        pt[:, lo:QCHUNK],
                        p_s[:, lo:QCHUNK],
                        mybir.ActivationFunctionType.Exp,
                        scale=SCALE,
                    )
                    if jt - qc * JPER >= 0:
                        nc.vector.tensor_mul(
                            pt[:, lo : lo + 128], pt[:, lo : lo + 128], tri
                        )
                    return pt

                # software pipeline: keep one S block in flight ahead of AV
                AHEAD = 2
                pts = {}
                for k in range(min(AHEAD, len(blocks))):
                    pts[k] = s_block(*blocks[k])
                for idx, (jt, lo) in enumerate(blocks):
                    if idx + AHEAD < len(blocks):
                        pts[idx + AHEAD] = s_block(*blocks[idx + AHEAD])
                    pt = pts.pop(idx)
                    nc.tensor.matmul(
                        p_out[:, lo:QCHUNK],
                        V[:, jt, :],
                        pt[:, lo:QCHUNK],
                        start=(jt == 0),
                        stop=(jt == n_jt - 1),
                    )

                # ---- normalize + transpose back to [q, d] ----
                oT = otp.tile([D + 1, QCHUNK], FP, tag="ot")
                nc.scalar.copy(oT, p_out)
                for i in range(JPER):
                    qt = qc * JPER + i
                    p_tr = ps_t.tile([128, D + 1], FP, tag="t")
                    nc.tensor.transpose(
                        p_tr,
                        oT[:, i * 128 : (i + 1) * 128],
                        ident[0 : D + 1, 0 : D + 1],
                    )
                    s_sb = otp.tile([128, 2], FP, tag="s_sb")
                    nc.vector.tensor_copy(s_sb[:, 0:1], p_tr[:, D : D + 1])
                    nc.vector.reciprocal(s_sb[:, 1:2], s_sb[:, 0:1])
                    nc.vector.tensor_scalar_mul(
                        out_sb[:, qt, :], p_tr[:, 0:D], s_sb[:, 1:2]
                    )

            nc.sync.dma_start(
                out=y_h[:, :].rearrange("(qt p) d -> p qt d", p=128), in_=out_sb
            )

    nc.finalize()
    return nc


_NC_CACHE = None
LAST_RESULTS = None


def kernel(x, Wq, Wk, Wv, trace=False, **run_kwargs):
    global _NC_CACHE, LAST_RESULTS
    x = np.ascontiguousarray(np.asarray(x, dtype=np.float32))
    wqk = np.ascontiguousarray(
        np.concatenate(
            [np.asarray(Wq, np.float32), np.asarray(Wk, np.float32)], axis=1
        )
    )
    wv = np.ascontiguousarray(np.asarray(Wv, dtype=np.float32))

    if _NC_CACHE is None:
        _NC_CACHE = build_nc()
    nc = _NC_CACHE

    in_maps = [
        {"xT": np.ascontiguousarray(x[b].T), "wqk": wqk, "wv": wv}
        for b in range(N_CORES)
    ]
    res = run_bass_kernel_spmd(
        nc, in_maps, core_ids=list(range(N_CORES)), trace=trace, **run_kwargs
    )
    LAST_RESULTS = res
    return np.stack([res.results[b]["y"] for b in range(N_CORES)], axis=0)


if __name__ == "__main__":
    rng = np.random.default_rng(0)
    x = rng.standard_normal((B, T, C), dtype=np.float32)
    s = 1.0 / np.sqrt(C)
    Wq = rng.standard_normal((C, D), dtype=np.float32) * s
    Wk = rng.standard_normal((C, D), dtype=np.float32) * s
    Wv = rng.standard_normal((C, D), dtype=np.float32) * s
    out = kernel(x, Wq, Wk, Wv)
    print("out", out.shape, out.dtype, float(np.abs(out).max()))



# revision 5
# speedup vs baseline: 1.6498x; 1.6498x over previous
"""Single-head causal self-attention on 8 Trainium2 NeuronCores.

Problem: x[8, 2048, 1024], Wq/Wk/Wv[1024, 64] ->
  out[b] = softmax(causal((x[b]@Wq) @ (x[b]@Wk)^T / 8)) @ (x[b]@Wv)

Sharding: data-parallel over batch B=8, one batch element per core; weights
replicated. All matmul operands are bf16 (1 PE cycle/row vs 4 for fp32, and
half the DMA bytes); accumulation stays fp32 in PSUM.

Per-core scheme:
  - host pre-packs x[b]^T as [128, 8, 2048] bf16 so every DMA line is long
    and contiguous per partition
  - [q^T;k^T] = Wqk^T @ x^T  (W-stationary, PSUM [128,512] per t-chunk,
    evacuated to qT/kT bf16 by DVE)
  - V = x @ Wv directly in natural [t, 64] layout (x-stationary matmuls:
    out cols = 64, so this costs half of the W-stationary form), plus a
    ones column -> V[128, 16, 65]
  - S^T[j-tile, q-chunk] = (k^T tile)^T @ q^T, causal blocks only; exp on
    ACT (PSUM->SBUF bf16); diagonal blocks masked with a bf16 triangle (DVE)
  - out[q-tile, 65] += P^T-block^T @ V[j]  (AV in natural layout: 65 output
    cols per block instead of 512 -> ~2x fewer PE cycles; col 64 is the
    softmax denominator for free)
  - normalize rows by reciprocal of col 64 (DVE), DMA out per chunk
  - a few warmup matmuls on junk data ramp the PE p-state to full clock
    while the first x chunk is still in flight
"""

import numpy as np
import ml_dtypes

import concourse.bass as bass
import concourse.mybir as mybir
import concourse.tile as tile
from concourse import bacc
from concourse.bass_utils import run_bass_kernel_spmd
from concourse.masks import make_upper_triangular

N_CORES = 8
B, T, C, D = 8, 2048, 1024, 64
CT = C // 128           # 8 contraction tiles
NT = T // 128           # 16 row tiles
QCHUNK = 512
NQC = T // QCHUNK       # 4 q-chunks
JPER = QCHUNK // 128    # 4 j-tiles per q-chunk
SCALE = float(1.0 / np.sqrt(D))
N_WARMUP = 8            # PE p-state ramp matmuls during initial DMA

FP = mybir.dt.float32
BF = mybir.dt.bfloat16


def build_nc():
    nc = bacc.Bacc("TRN2", target_bir_lowering=False)
    xT_h = nc.dram_tensor("xT", [128, CT, T], BF, kind="ExternalInput")
    wqk_h = nc.dram_tensor("wqk", [128, CT, 128], BF, kind="ExternalInput")
    wv_h = nc.dram_tensor("wv", [128, CT, D], BF, kind="ExternalInput")
    y_h = nc.dram_tensor("y", [128, NT, D], FP, kind="ExternalOutput")

    with tile.TileContext(nc) as tc:
        with (
            tc.tile_pool(name="const", bufs=1) as const,
            tc.tile_pool(name="pt", bufs=6) as ptp,
            tc.tile_pool(name="dve", bufs=2) as dvp,
            tc.tile_pool(name="ps_qk", bufs=1, space="PSUM") as ps_qk,
            tc.tile_pool(name="ps_v", bufs=1, space="PSUM") as ps_v,
            tc.tile_pool(name="ps_s", bufs=3, space="PSUM") as ps_s,
            tc.tile_pool(name="ps_o", bufs=2, space="PSUM") as ps_o,
            tc.tile_pool(name="ps_w", bufs=1, space="PSUM") as ps_w,
        ):
            # ---- constants ----
            tri = const.tile([128, 128], BF, tag="tri")  # tri[p,f]=1.0 iff f>=p
            make_upper_triangular(nc, tri, val=1.0, diag=True)

            wu = const.tile([128, QCHUNK], BF, tag="wu")
            nc.gpsimd.memset(wu, 0.0)

            wqk_sb = const.tile([128, CT, 128], BF, tag="wqk")
            nc.sync.dma_start(out=wqk_sb, in_=wqk_h[:, :, :])
            wv_sb = const.tile([128, CT, D], BF, tag="wv")
            nc.sync.dma_start(out=wv_sb, in_=wv_h[:, :, :])

            xT_sb = const.tile([128, CT, T], BF, tag="xT")
            qT = const.tile([64, T], BF, tag="qT")
            kT = const.tile([64, T], BF, tag="kT")
            V = const.tile([128, NT, D + 1], BF, tag="V")  # col D = ones
            nc.gpsimd.memset(V[:, :, D], 1.0)
            out_sb = const.tile([128, NT, D], FP, tag="out")

            # ---- PE p-state warmup while the first DMA is in flight ----
            ps_junk = ps_w.tile([128, QCHUNK], FP, tag="junk")
            for w in range(N_WARMUP):
                nc.tensor.matmul(
                    ps_junk, wu[:, 0:128], wu, start=True, stop=True
                )

            # ---- chunk-pipelined: DMA, projections, attention ----
            for cu in range(NQC):
                sl = slice(cu * QCHUNK, (cu + 1) * QCHUNK)
                nc.sync.dma_start(
                    out=xT_sb[:, 0 : CT // 2, sl], in_=xT_h[:, 0 : CT // 2, sl]
                )
                nc.sync.dma_start(
                    out=xT_sb[:, CT // 2 : CT, sl], in_=xT_h[:, CT // 2 : CT, sl]
                )

                # q,k projection: W-stationary, psum rows 0:64=q / 64:128=k
                p_qk = ps_qk.tile([128, QCHUNK], FP, tag="qk")
                for ct in range(CT):
                    nc.tensor.matmul(
                        p_qk,
                        wqk_sb[:, ct, :],
                        xT_sb[:, ct, sl],
                        start=(ct == 0),
                        stop=(ct == CT - 1),
                    )
                nc.vector.tensor_copy(qT[:, sl], p_qk[0:64, :])
                nc.scalar.copy(kT[:, sl], p_qk[64:128, :])  # partition shift

                # v projection: x-stationary -> natural V layout (64-col outs)
                p_v = ps_v.tile([128, JPER, D], FP, tag="v")
                # PSUM `start` zeroes the whole 2KB bank (zero region), so only
                # the first matmul into the tile starts; later regions get
                # fresh-write via the pending-zero mechanics.
                for tt in range(JPER):
                    tsl = slice(cu * QCHUNK + tt * 128, cu * QCHUNK + (tt + 1) * 128)
                    for ct in range(CT):
                        nc.tensor.matmul(
                            p_v[:, tt, :],
                            xT_sb[:, ct, tsl],
                            wv_sb[:, ct, :],
                            start=(tt == 0 and ct == 0),
                            stop=(tt == JPER - 1 and ct == CT - 1),
                            skip_group_check=True,
                        )
                nc.vector.tensor_copy(V[:, cu * JPER : (cu + 1) * JPER, 0:D], p_v)

                # ---- attention for q-chunk cu (k/v tiles j = 0..4cu+3) ----
                n_jt = cu * JPER + JPER
                p_out = ps_o.tile([128, JPER, D + 1], FP, tag="o")

                def s_block(j):
                    i_d = j - cu * JPER
                    lo = max(i_d, 0) * 128
                    p_s = ps_s.tile([128, QCHUNK], FP, tag="s")
                    nc.tensor.matmul(
                        p_s[:, lo:QCHUNK],
                        kT[:, j * 128 : (j + 1) * 128],
                        qT[:, cu * QCHUNK + lo : (cu + 1) * QCHUNK],
                        start=True,
                        stop=True,
                    )
                    pt = ptp.tile([128, QCHUNK], BF, tag="pt")
                    nc.scalar.activation(
                        pt[:, lo:QCHUNK],
                        p_s[:, lo:QCHUNK],
                        mybir.ActivationFunctionType.Exp,
                        scale=SCALE,
                    )
                    if i_d >= 0:
                        nc.vector.tensor_mul(
                            pt[:, lo : lo + 128], pt[:, lo : lo + 128], tri
                        )
                    return pt

                def av_block(j, pt):
                    i_d = j - cu * JPER
                    for i in range(max(i_d, 0), JPER):
                        nc.tensor.matmul(
                            p_out[:, i, :],
                            pt[:, i * 128 : (i + 1) * 128],
                            V[:, j, :],
                            start=(j == 0 and i == 0),
                            stop=(j == n_jt - 1 and i == JPER - 1),
                            skip_group_check=True,
                        )

                AHEAD = 3
                pts = {}
                for j in range(min(AHEAD, n_jt)):
                    pts[j] = s_block(j)
                for j in range(n_jt):
                    if j + AHEAD < n_jt:
                        pts[j + AHEAD] = s_block(j + AHEAD)
                    av_block(j, pts.pop(j))

                # ---- normalize by the ones-column sum and store ----
                rec = dvp.tile([128, JPER], FP, tag="rec")
                nc.vector.reciprocal(rec, p_out[:, :, D])
                for i in range(JPER):
                    nc.vector.tensor_scalar_mul(
                        out_sb[:, cu * JPER + i, :],
                        p_out[:, i, 0:D],
                        rec[:, i : i + 1],
                    )
                nc.sync.dma_start(
                    out=y_h[:, cu * JPER : (cu + 1) * JPER, :],
                    in_=out_sb[:, cu * JPER : (cu + 1) * JPER, :],
                )

    nc.finalize()
    return nc


_NC_CACHE = None
LAST_RESULTS = None


def _pack(w, cols):
    # [C, cols] -> [128, CT, cols] with partition p holding rows {ct*128+p}
    return np.ascontiguousarray(
        np.asarray(w, np.float32).reshape(CT, 128, cols).transpose(1, 0, 2)
    ).astype(ml_dtypes.bfloat16)


def kernel(x, Wq, Wk, Wv, trace=False, **run_kwargs):
    global _NC_CACHE, LAST_RESULTS
    x = np.asarray(x, dtype=np.float32)
    wqk = _pack(np.concatenate([np.asarray(Wq, np.float32),
                                np.asarray(Wk, np.float32)], axis=1), 128)
    wv = _pack(Wv, D)

    if _NC_CACHE is None:
        _NC_CACHE = build_nc()
    nc = _NC_CACHE

    in_maps = []
    for b in range(N_CORES):
        # xT[p, ct, t] = x[b, t, ct*128+p]
        xT = np.ascontiguousarray(
            x[b].T.reshape(CT, 128, T).transpose(1, 0, 2)
        ).astype(ml_dtypes.bfloat16)
        in_maps.append({"xT": xT, "wqk": wqk, "wv": wv})

    res = run_bass_kernel_spmd(
        nc, in_maps, core_ids=list(range(N_CORES)), trace=trace, **run_kwargs
    )
    LAST_RESULTS = res
    out = np.empty((N_CORES, T, D), dtype=np.float32)
    for b in range(N_CORES):
        y = res.results[b]["y"]  # [128, NT, D]
        out[b] = np.asarray(y, dtype=np.float32).transpose(1, 0, 2).reshape(T, D)
    return out


if __name__ == "__main__":
    rng = np.random.default_rng(0)
    x = rng.standard_normal((B, T, C), dtype=np.float32)
    s = 1.0 / np.sqrt(C)
    Wq = rng.standard_normal((C, D), dtype=np.float32) * s
    Wk = rng.standard_normal((C, D), dtype=np.float32) * s
    Wv = rng.standard_normal((C, D), dtype=np.float32) * s
    out = kernel(x, Wq, Wk, Wv)
    print("out", out.shape, out.dtype, float(np.abs(out).max()))


# revision 30
# speedup vs baseline: 1.8259x; 1.1067x over previous
"""Single-head causal self-attention on 8 Trainium2 NeuronCores.

Problem: x[8, 2048, 1024], Wq/Wk/Wv[1024, 64] ->
  out[b] = softmax(causal((x[b]@Wq) @ (x[b]@Wk)^T / 8)) @ (x[b]@Wv)

Sharding: data-parallel over batch B=8, one batch element per core; weights
replicated. All matmul operands are bf16 (1 PE cycle/row vs 4 for fp32, and
half the DMA bytes); accumulation stays fp32 in PSUM.

Per-core scheme:
  - host pre-packs x[b]^T as [128, 8, 2048] bf16 so every DMA line is long
    and contiguous per partition; input DMAs are issued upfront on SP in the
    order compute consumes them (wqk, x0, wv, x1, x2, x3)
  - [q^T;k^T] = Wqk^T @ x^T  (W-stationary, PSUM [128,512] per t-chunk; q
    half evacuated by DVE, k half - which needs a partition shift - by the
    scalar engine early on / DVE later); V = x @ Wv in natural [t, 64]
    layout (x-stationary: 64-col outputs, half the PE cycles of the
    W-stationary form), accumulated into the same PSUM bank after the qk
    halves are evacuated
  - S^T[j-tile, q-chunk] = (k^T tile)^T @ q^T, causal blocks only;
    off-diagonal j-tiles are computed in PAIRS into a 2-bank PSUM tile so a
    single ACT exp instruction covers 1024 columns (halves ACT's fixed
    per-instruction access overhead); diagonal tiles stay single, sliced at
    the causal boundary, and are masked with a bf16 triangle on DVE
  - out[q-tile, 65] += P^T-block^T @ V[j]  (AV in natural layout: 65 output
    cols per block; col 64 of V is ones, making the softmax denominator a
    free by-product); rows normalized with DVE reciprocal (per-tile for the
    last chunk to shorten the drain)
  - attention units from ALL chunks form one software-pipelined stream; the
    next chunk's projections are emitted between units so neither PE nor ACT
    drains at chunk boundaries (engines execute strictly in emission order)
  - warmup matmuls on junk data ramp the PE p-state to full clock while the
    first x chunk is in flight; the Exp table is preloaded at t~0
"""

import numpy as np
import ml_dtypes

import concourse.bass as bass
import concourse.mybir as mybir
import concourse.tile as tile
from concourse import bacc
from concourse.bass_utils import run_bass_kernel_spmd
from concourse.masks import make_upper_triangular

N_CORES = 8
B, T, C, D = 8, 2048, 1024, 64
CT = C // 128           # 8 contraction tiles
NT = T // 128           # 16 row tiles
QCHUNK = 512
NQC = T // QCHUNK       # 4 q-chunks
JPER = QCHUNK // 128    # 4 j-tiles per q-chunk
SCALE = float(1.0 / np.sqrt(D))
N_WARMUP = 34           # 128-col PE p-state ramp matmuls during initial DMA

FP = mybir.dt.float32
BF = mybir.dt.bfloat16


def build_nc():
    nc = bacc.Bacc("TRN2", target_bir_lowering=False)
    xT_h = nc.dram_tensor("xT", [128, CT, T], BF, kind="ExternalInput")
    wqk_h = nc.dram_tensor("wqk", [128, CT, 128], BF, kind="ExternalInput")
    wv_h = nc.dram_tensor("wv", [128, CT, D], BF, kind="ExternalInput")
    y_h = nc.dram_tensor("y", [128, NT, D], FP, kind="ExternalOutput")

    with tile.TileContext(nc) as tc:
        with (
            tc.tile_pool(name="const", bufs=1) as const,
            tc.tile_pool(name="pt", bufs=3) as ptp,      # [128,2,512] bf16
            tc.tile_pool(name="dve", bufs=2) as dvp,
            tc.tile_pool(name="ps_s2", bufs=2, space="PSUM") as ps_s2,  # 2x2 banks
            tc.tile_pool(name="ps_pq", bufs=1, space="PSUM") as ps_pq,  # 1 bank
            tc.tile_pool(name="ps_pv", bufs=1, space="PSUM") as ps_pv,  # 1 bank
            tc.tile_pool(name="ps_o", bufs=2, space="PSUM") as ps_o,    # 2 banks
        ):
            # ---- constants (wu first: it gates the PE warmup) ----
            wu = const.tile([128, 128], BF, tag="wu")
            nc.gpsimd.memset(wu, 0.0)
            tri = const.tile([128, 128], BF, tag="tri")  # tri[p,f]=1.0 iff f>=p
            make_upper_triangular(nc, tri, val=1.0, diag=True)
            # preload the Exp activation table while DMAs are in flight
            dum = const.tile([1, 2], BF, tag="dum")
            nc.scalar.activation(
                dum[0:1, 0:1], wu[0:1, 0:1], mybir.ActivationFunctionType.Exp
            )

            wqk_sb = const.tile([128, CT, 128], BF, tag="wqk")
            wv_sb = const.tile([128, CT, D], BF, tag="wv")
            xT_sb = const.tile([128, CT, T], BF, tag="xT")

            # input DMAs, all upfront on SP, in consumption order
            def xdma(cu):
                sl = slice(cu * QCHUNK, (cu + 1) * QCHUNK)
                nc.sync.dma_start(
                    out=xT_sb[:, 0 : CT // 2, sl], in_=xT_h[:, 0 : CT // 2, sl]
                )
                nc.sync.dma_start(
                    out=xT_sb[:, CT // 2 : CT, sl], in_=xT_h[:, CT // 2 : CT, sl]
                )

            nc.sync.dma_start(out=wqk_sb, in_=wqk_h[:, :, :])
            xdma(0)
            nc.sync.dma_start(out=wv_sb, in_=wv_h[:, :, :])
            for cu in range(1, NQC):
                xdma(cu)

            # q/k live on partitions 64:128 (psum high half evacuates with no
            # partition shift); k's low half goes through a staging tile and an
            # SBUF->SBUF DMA (the only engine-free way to shift partitions)
            qT = const.tile([128, T], BF, tag="qT")
            kT = const.tile([128, T], BF, tag="kT")
            kst = const.tile([64, T], BF, tag="kst")
            V = const.tile([128, NT, D + 1], BF, tag="V")  # col D = ones
            nc.gpsimd.memset(V[:, :, D], 1.0)
            out_sb = const.tile([128, NT, D], FP, tag="out")

            # ---- PE p-state warmup while the first DMA is in flight ----
            ps_junk = ps_s2.tile([128, 2, QCHUNK], FP, tag="s2")
            for w in range(N_WARMUP):
                nc.tensor.matmul(
                    ps_junk[:, 0, 0:128], wu, wu, start=True, stop=True
                )

            proj_psum = {}

            def proj(cu):
                """qk (W-stationary) into its own bank; v (x-stationary) into a
                separate bank so it does not WAR-wait on the qT/kT copies."""
                sl = slice(cu * QCHUNK, (cu + 1) * QCHUNK)
                p_qk = ps_pq.tile([128, QCHUNK], FP, tag="pq")
                for ct in range(CT):
                    nc.tensor.matmul(
                        p_qk,
                        wqk_sb[:, ct, :],
                        xT_sb[:, ct, sl],
                        start=(ct == 0),
                        stop=(ct == CT - 1),
                    )
                p_v = ps_pv.tile([128, JPER, D], FP, tag="pv")
                for tt in range(JPER):
                    tsl = slice(cu * QCHUNK + tt * 128, cu * QCHUNK + (tt + 1) * 128)
                    for ct in range(CT):
                        nc.tensor.matmul(
                            p_v[:, tt, :],
                            xT_sb[:, ct, tsl],
                            wv_sb[:, ct, :],
                            start=(tt == 0 and ct == 0),
                            stop=(tt == JPER - 1 and ct == CT - 1),
                            skip_group_check=True,
                        )
                nc.vector.tensor_copy(qT[64:128, sl], p_qk[64:128, :])
                if cu == 0:
                    # head: ACT is idle, and the direct shifted copy is the
                    # lowest-latency path for the very first S unit
                    nc.scalar.copy(kT[64:128, sl], p_qk[0:64, :])
                else:
                    nc.vector.tensor_copy(kst[:, sl], p_qk[0:64, :])
                    nc.gpsimd.dma_start(out=kT[64:128, sl], in_=kst[:, sl])
                proj_psum[cu] = p_v

            def v_copy(cu):
                p_v = proj_psum.pop(cu)
                nc.vector.tensor_copy(V[:, cu * JPER : (cu + 1) * JPER, 0:D], p_v)

            # ---- global attention unit stream: all units are j-tile PAIRS --
            units = []
            for cu in range(NQC):
                for j in range(0, (cu + 1) * JPER, 2):
                    units.append((cu, (j, j + 1)))
            first_unit = {cu: next(i for i, (c, _) in enumerate(units) if c == cu)
                          for cu in range(NQC)}
            last_unit = {cu: max(i for i, (c, _) in enumerate(units) if c == cu)
                         for cu in range(NQC)}
            # emit proj(cu+1) after this many units of chunk cu (timed so the
            # PE's in-order queue never parks on a not-yet-arrived x chunk);
            # must fire before k reaches first_unit[cu+1] - an s_unit can only
            # be emitted after its chunk's proj (its qT/kT must have a writer)
            proj_trigger = {last_unit[0]: 1, first_unit[1] + 2: 2,
                            first_unit[2] + 4: 3}

            p_outs = {}

            def s_unit(i):
                cu, u = units[i]
                # both tiles of the pair are computed from the pair's lowest
                # causal column so a single exp AP covers them; columns left
                # of a tile's own boundary are never read by AV
                lo = max(u[0] - cu * JPER, 0) * 128
                p_s = ps_s2.tile([128, 2, QCHUNK], FP, tag="s2")
                pt = ptp.tile([128, 2, QCHUNK], BF, tag="pt")
                for z, j in enumerate(u):
                    nc.tensor.matmul(
                        p_s[:, z, lo:QCHUNK],
                        kT[64:128, j * 128 : (j + 1) * 128],
                        qT[64:128, cu * QCHUNK + lo : (cu + 1) * QCHUNK],
                        start=True,
                        stop=True,
                    )
                nc.scalar.activation(
                    pt[:, :, lo:QCHUNK],
                    p_s[:, :, lo:QCHUNK],
                    mybir.ActivationFunctionType.Exp,
                    scale=SCALE,
                )
                for z, j in enumerate(u):
                    i_d = j - cu * JPER
                    if i_d >= 0:
                        nc.vector.tensor_mul(
                            pt[:, z, i_d * 128 : (i_d + 1) * 128],
                            pt[:, z, i_d * 128 : (i_d + 1) * 128],
                            tri,
                        )
                return pt

            def av_unit(i, pt):
                cu, u = units[i]
                n_jt = cu * JPER + JPER
                if i == first_unit[cu]:
                    v_copy(cu)
                    p_outs[cu] = ps_o.tile([128, JPER, D + 1], FP, tag="o", name=f"p_out{cu}")
                p_out = p_outs[cu]
                for z, j in enumerate(u):
                    pj = pt[:, z, :]
                    i_d = j - cu * JPER
                    for qi in range(max(i_d, 0), JPER):
                        nc.tensor.matmul(
                            p_out[:, qi, :],
                            pj[:, qi * 128 : (qi + 1) * 128],
                            V[:, j, :],
                            start=(j == 0 and qi == 0),
                            stop=(j == n_jt - 1 and qi == JPER - 1),
                            skip_group_check=True,
                        )
                # last chunk: tiles 0..1 are complete after the (12,13) pair;
                # only tiles 2..3's normalize + DMA trail the final AV
                if cu == NQC - 1 and i == last_unit[cu] - 1:
                    finalize(cu, 0, 2)
                elif i == last_unit[cu]:
                    if cu == NQC - 1:
                        finalize(cu, 2, JPER)
                    else:
                        finalize(cu, 0, JPER)

            def finalize(cu, lo, hi):
                p_out = p_outs[cu]
                rec = dvp.tile([128, hi - lo], FP, tag=f"rec{hi - lo}")
                nc.vector.reciprocal(rec, p_out[:, lo:hi, D])
                for qi in range(lo, hi):
                    nc.vector.tensor_scalar_mul(
                        out_sb[:, cu * JPER + qi, :],
                        p_out[:, qi, 0:D],
                        rec[:, qi - lo : qi - lo + 1],
                    )
                nc.sync.dma_start(
                    out=y_h[:, cu * JPER + lo : cu * JPER + hi, :],
                    in_=out_sb[:, cu * JPER + lo : cu * JPER + hi, :],
                )

            # ---- software pipeline over the global unit stream ----
            AHEAD = 2
            nu = len(units)
            pts = {}
            projs_done = set()
            state = {"s": 0}

            def pump_s(upto):
                # emit s_units up to index `upto`, but never before the
                # owning chunk's projection exists (its qT/kT writer)
                while state["s"] < min(upto, nu):
                    if units[state["s"]][0] not in projs_done:
                        break
                    pts[state["s"]] = s_unit(state["s"])
                    state["s"] += 1

            proj(0)
            projs_done.add(0)
            pump_s(AHEAD)
            for k in range(nu):
                pump_s(k + 1 + AHEAD)
                assert state["s"] > k, f"s_unit({k}) blocked: proj not emitted"
                av_unit(k, pts.pop(k))
                if k in proj_trigger:
                    proj(proj_trigger[k])
                    projs_done.add(proj_trigger[k])
                    pump_s(k + 1 + AHEAD)

    nc.finalize()
    return nc


_NC_CACHE = None
LAST_RESULTS = None


def _pack(w, cols):
    # [C, cols] -> [128, CT, cols] with partition p holding rows {ct*128+p}
    return np.ascontiguousarray(
        np.asarray(w, np.float32).reshape(CT, 128, cols).transpose(1, 0, 2)
    ).astype(ml_dtypes.bfloat16)


def kernel(x, Wq, Wk, Wv, trace=False, **run_kwargs):
    global _NC_CACHE, LAST_RESULTS
    x = np.asarray(x, dtype=np.float32)
    # k in the psum low half, q in the high half (see build_nc)
    wqk = _pack(np.concatenate([np.asarray(Wk, np.float32),
                                np.asarray(Wq, np.float32)], axis=1), 128)
    wv = _pack(Wv, D)

    if _NC_CACHE is None:
        _NC_CACHE = build_nc()
    nc = _NC_CACHE

    in_maps = []
    for b in range(N_CORES):
        # xT[p, ct, t] = x[b, t, ct*128+p]
        xT = np.ascontiguousarray(
            x[b].T.reshape(CT, 128, T).transpose(1, 0, 2)
        ).astype(ml_dtypes.bfloat16)
        in_maps.append({"xT": xT, "wqk": wqk, "wv": wv})

    res = run_bass_kernel_spmd(
        nc, in_maps, core_ids=list(range(N_CORES)), trace=trace, **run_kwargs
    )
    LAST_RESULTS = res
    out = np.empty((N_CORES, T, D), dtype=np.float32)
    for b in range(N_CORES):
        y = res.results[b]["y"]  # [128, NT, D]
        out[b] = np.asarray(y, dtype=np.float32).transpose(1, 0, 2).reshape(T, D)
    return out


if __name__ == "__main__":
    rng = np.random.default_rng(0)
    x = rng.standard_normal((B, T, C), dtype=np.float32)
    s = 1.0 / np.sqrt(C)
    Wq = rng.standard_normal((C, D), dtype=np.float32) * s
    Wk = rng.standard_normal((C, D), dtype=np.float32) * s
    Wv = rng.standard_normal((C, D), dtype=np.float32) * s
    out = kernel(x, Wq, Wk, Wv)
    print("out", out.shape, out.dtype, float(np.abs(out).max()))


# revision 46
# speedup vs baseline: 1.8394x; 1.0074x over previous
"""Single-head causal self-attention on 8 Trainium2 NeuronCores.

Problem: x[8, 2048, 1024], Wq/Wk/Wv[1024, 64] ->
  out[b] = softmax(causal((x[b]@Wq) @ (x[b]@Wk)^T / 8)) @ (x[b]@Wv)

Sharding: data-parallel over batch B=8, one batch element per core; weights
replicated. All matmul operands are bf16 (1 PE cycle/row vs 4 for fp32, and
half the DMA bytes); accumulation stays fp32 in PSUM.

Per-core scheme:
  - host pre-packs x[b]^T as [128, 8, 2048] bf16 so every DMA line is long
    and contiguous per partition; input DMAs are issued upfront on SP in the
    order compute consumes them (wqk, x0, wv, x1, x2, x3)
  - [q^T;k^T] = Wqk^T @ x^T  (W-stationary, PSUM [128,512] per t-chunk; q
    half evacuated by DVE, k half - which needs a partition shift - by the
    scalar engine early on / DVE later); V = x @ Wv in natural [t, 64]
    layout (x-stationary: 64-col outputs, half the PE cycles of the
    W-stationary form), accumulated into the same PSUM bank after the qk
    halves are evacuated
  - S^T[j-tile, q-chunk] = (k^T tile)^T @ q^T, causal blocks only;
    off-diagonal j-tiles are computed in PAIRS into a 2-bank PSUM tile so a
    single ACT exp instruction covers 1024 columns (halves ACT's fixed
    per-instruction access overhead); diagonal tiles stay single, sliced at
    the causal boundary, and are masked with a bf16 triangle on DVE
  - out[q-tile, 65] += P^T-block^T @ V[j]  (AV in natural layout: 65 output
    cols per block; col 64 of V is ones, making the softmax denominator a
    free by-product); rows normalized with DVE reciprocal (per-tile for the
    last chunk to shorten the drain)
  - attention units from ALL chunks form one software-pipelined stream; the
    next chunk's projections are emitted between units so neither PE nor ACT
    drains at chunk boundaries (engines execute strictly in emission order)
  - warmup matmuls on junk data ramp the PE p-state to full clock while the
    first x chunk is in flight; the Exp table is preloaded at t~0
"""

import numpy as np
import ml_dtypes

import concourse.bass as bass
import concourse.mybir as mybir
import concourse.tile as tile
from concourse import bacc
from concourse.bass_utils import run_bass_kernel_spmd
from concourse.masks import make_upper_triangular

N_CORES = 8
B, T, C, D = 8, 2048, 1024, 64
CT = C // 128           # 8 contraction tiles
NT = T // 128           # 16 row tiles
QCHUNK = 512
NQC = T // QCHUNK       # 4 q-chunks
JPER = QCHUNK // 128    # 4 j-tiles per q-chunk
SCALE = float(1.0 / np.sqrt(D))
N_WARMUP = 34           # 128-col PE p-state ramp matmuls during initial DMA

FP = mybir.dt.float32
BF = mybir.dt.bfloat16


def build_nc():
    nc = bacc.Bacc("TRN2", target_bir_lowering=False)
    xT_h = nc.dram_tensor("xT", [128, CT, T], BF, kind="ExternalInput")
    wqk_h = nc.dram_tensor("wqk", [128, CT, 128], BF, kind="ExternalInput")
    wv_h = nc.dram_tensor("wv", [128, CT, D], BF, kind="ExternalInput")
    y_h = nc.dram_tensor("y", [128, NT, D], FP, kind="ExternalOutput")

    with tile.TileContext(nc) as tc:
        with (
            tc.tile_pool(name="const", bufs=1) as const,
            tc.tile_pool(name="pt", bufs=6) as ptp,      # [128,2,512] bf16
            tc.tile_pool(name="dve", bufs=2) as dvp,
            tc.tile_pool(name="ps_s2", bufs=2, space="PSUM") as ps_s2,  # 2x2 banks
            tc.tile_pool(name="ps_pq", bufs=2, space="PSUM") as ps_pq,  # 2 banks
            tc.tile_pool(name="ps_pv", bufs=1, space="PSUM") as ps_pv,  # 1 bank
            tc.tile_pool(name="ps_o", bufs=1, space="PSUM") as ps_o,    # 1 bank
        ):
            # ---- constants (wu first: it gates the PE warmup) ----
            wu = const.tile([128, 128], BF, tag="wu")
            nc.gpsimd.memset(wu, 0.0)
            tri = const.tile([128, 128], BF, tag="tri")  # tri[p,f]=1.0 iff f>=p
            make_upper_triangular(nc, tri, val=1.0, diag=True)
            # preload the Exp activation table while DMAs are in flight
            dum = const.tile([1, 2], BF, tag="dum")
            nc.scalar.activation(
                dum[0:1, 0:1], wu[0:1, 0:1], mybir.ActivationFunctionType.Exp
            )

            wqk_sb = const.tile([128, CT, 128], BF, tag="wqk")
            wv_sb = const.tile([128, CT, D], BF, tag="wv")
            xT_sb = const.tile([128, CT, T], BF, tag="xT")

            # input DMAs, all upfront on SP, in consumption order; ct-quarters
            # let the qk contraction start on the first arriving piece
            def xdma(cu):
                sl = slice(cu * QCHUNK, (cu + 1) * QCHUNK)
                for q in range(4):
                    nc.sync.dma_start(
                        out=xT_sb[:, 2 * q : 2 * q + 2, sl],
                        in_=xT_h[:, 2 * q : 2 * q + 2, sl],
                    )

            nc.sync.dma_start(out=wqk_sb, in_=wqk_h[:, :, :])
            xdma(0)
            nc.sync.dma_start(out=wv_sb, in_=wv_h[:, :, :])
            for cu in range(1, NQC):
                xdma(cu)

            # q/k live on partitions 64:128 (psum high half evacuates with no
            # partition shift); k's low half goes through a staging tile and an
            # SBUF->SBUF DMA (the only engine-free way to shift partitions)
            qT = const.tile([128, T], BF, tag="qT")
            kT = const.tile([128, T], BF, tag="kT")
            kst = const.tile([64, T], BF, tag="kst")
            V = const.tile([128, NT, D + 1], BF, tag="V")  # col D = ones
            nc.gpsimd.memset(V[:, :, D], 1.0)
            out_sb = const.tile([128, NT, D], FP, tag="out")

            # ---- PE p-state warmup while the first DMA is in flight ----
            ps_junk = ps_s2.tile([128, 2, QCHUNK], FP, tag="s2")
            for w in range(N_WARMUP):
                nc.tensor.matmul(
                    ps_junk[:, 0, 0:128], wu, wu, start=True, stop=True
                )

            proj_psum = {}

            proj_psums = {}

            def proj_qk(cu, lo_ct, hi_ct):
                """qk (W-stationary), emitted in ct-halves so the PE order can
                match the x-quarter DMA arrivals; on the last half, q/k are
                evacuated to bf16: one full-width DVE copy (q rows 64: feed S
                directly), k rows :64 partition-shifted into kT by an
                SBUF->SBUF DMA on the idle Pool queue."""
                sl = slice(cu * QCHUNK, (cu + 1) * QCHUNK)
                if lo_ct == 0:
                    proj_psums[cu] = ps_pq.tile(
                        [128, QCHUNK], FP, tag="pq", name=f"p_qk{cu}"
                    )
                p_qk = proj_psums[cu]
                for ct in range(lo_ct, hi_ct):
                    nc.tensor.matmul(
                        p_qk,
                        wqk_sb[:, ct, :],
                        xT_sb[:, ct, sl],
                        start=(ct == 0),
                        stop=(ct == CT - 1),
                    )
                if hi_ct < CT:
                    return
                nc.vector.tensor_copy(qT[:, sl], p_qk)
                if cu < NQC - 1:
                    # ACT has boundary idle here, and the direct shifted copy
                    # is the lowest-latency path to kT
                    nc.scalar.copy(kT[64:128, sl], p_qk[0:64, :])
                else:
                    # last chunk: ACT is saturated, but the DMA engines are
                    # free (all x transfers done) - shift via SBUF->SBUF DMA
                    nc.gpsimd.dma_start(out=kT[64:128, sl], in_=qT[0:64, sl])

            def proj_v(cu):
                """v (x-stationary, natural layout) + evacuation into V."""
                p_v = ps_pv.tile([128, JPER, D], FP, tag="pv")
                for tt in range(JPER):
                    tsl = slice(cu * QCHUNK + tt * 128, cu * QCHUNK + (tt + 1) * 128)
                    for ct in range(CT):
                        nc.tensor.matmul(
                            p_v[:, tt, :],
                            xT_sb[:, ct, tsl],
                            wv_sb[:, ct, :],
                            start=(tt == 0 and ct == 0),
                            stop=(tt == JPER - 1 and ct == CT - 1),
                            skip_group_check=True,
                        )
                nc.vector.tensor_copy(V[:, cu * JPER : (cu + 1) * JPER, 0:D], p_v)

            # ---- global attention unit stream: all units are j-tile PAIRS --
            # unit key (cu, j0): j-tiles (j0, j0+1) against q-chunk cu
            units = {}
            for cu in range(NQC):
                for j in range(0, (cu + 1) * JPER, 2):
                    units[(cu, j)] = (cu, (j, j + 1))

            p_outs = {}

            def s_unit(key):
                cu, u = units[key]
                # both tiles of the pair are computed from the pair's lowest
                # causal column so a single exp AP covers them; columns left
                # of a tile's own boundary are never read by AV
                lo = max(u[0] - cu * JPER, 0) * 128
                p_s = ps_s2.tile([128, 2, QCHUNK], FP, tag="s2")
                pt = ptp.tile([128, 2, QCHUNK], BF, tag="pt")
                for z, j in enumerate(u):
                    nc.tensor.matmul(
                        p_s[:, z, lo:QCHUNK],
                        kT[64:128, j * 128 : (j + 1) * 128],
                        qT[64:128, cu * QCHUNK + lo : (cu + 1) * QCHUNK],
                        start=True,
                        stop=True,
                    )
                nc.scalar.activation(
                    pt[:, :, lo:QCHUNK],
                    p_s[:, :, lo:QCHUNK],
                    mybir.ActivationFunctionType.Exp,
                    scale=SCALE,
                )
                for z, j in enumerate(u):
                    i_d = j - cu * JPER
                    if i_d >= 0:
                        nc.vector.tensor_mul(
                            pt[:, z, i_d * 128 : (i_d + 1) * 128],
                            pt[:, z, i_d * 128 : (i_d + 1) * 128],
                            tri,
                        )
                return pt

            def av_unit(key, pt):
                cu, u = units[key]
                n_jt = cu * JPER + JPER
                if key[1] == 0:
                    p_outs[cu] = ps_o.tile([128, JPER, D + 1], FP, tag="o", name=f"p_out{cu}")
                p_out = p_outs[cu]
                for z, j in enumerate(u):
                    pj = pt[:, z, :]
                    i_d = j - cu * JPER
                    for qi in range(max(i_d, 0), JPER):
                        nc.tensor.matmul(
                            p_out[:, qi, :],
                            pj[:, qi * 128 : (qi + 1) * 128],
                            V[:, j, :],
                            start=(j == 0 and qi == 0),
                            stop=(j == n_jt - 1 and qi == JPER - 1),
                            skip_group_check=True,
                        )

            def finalize(cu, lo, hi):
                p_out = p_outs[cu]
                last = cu == NQC - 1
                rec = dvp.tile([128, hi - lo], FP, tag=f"rec{hi - lo}{last}")
                nc.vector.reciprocal(rec, p_out[:, lo:hi, D])
                for qi in range(lo, hi):
                    if last and qi == lo:
                        # drain: split the final normalizes across ACT + DVE
                        nc.scalar.mul(
                            out_sb[:, cu * JPER + qi, :],
                            p_out[:, qi, 0:D],
                            rec[:, qi - lo : qi - lo + 1],
                        )
                    else:
                        nc.vector.tensor_scalar_mul(
                            out_sb[:, cu * JPER + qi, :],
                            p_out[:, qi, 0:D],
                            rec[:, qi - lo : qi - lo + 1],
                        )
                nc.sync.dma_start(
                    out=y_h[:, cu * JPER + lo : cu * JPER + hi, :],
                    in_=out_sb[:, cu * JPER + lo : cu * JPER + hi, :],
                )

            # ---- explicit hand-scheduled action stream -------------------
            # Engines execute in emission order, so cross-chunk interleaving
            # is encoded directly: diagonal units (which wait on the staged
            # kT DMA) are deferred behind later chunks' off-diagonal units;
            # qk halves are placed where their x quarters have landed.
            S, AV = "s", "av"
            actions = [
                ("qk", 0, 0, 8), ("v", 0),
                ("qk", 1, 0, 2), (S, 0, 0), ("qk", 1, 2, 4), (S, 0, 2),
                ("qk", 1, 4, 6), ("qk", 1, 6, 8),
                (AV, 0, 0), (AV, 0, 2), ("fin", 0, 0, 4),
                ("qk", 2, 0, 2), ("v", 1),
                (S, 1, 0), (S, 1, 2),
                ("qk", 2, 2, 4), ("qk", 2, 4, 6),
                (AV, 1, 0), (AV, 1, 2),
                ("qk", 2, 6, 8),
                (S, 2, 0),
                ("qk", 3, 0, 2), ("v", 2),
                (S, 1, 4), (S, 2, 2),
                ("qk", 3, 2, 4),
                (AV, 1, 4),
                ("qk", 3, 4, 6),
                (S, 1, 6), (S, 2, 4),
                ("qk", 3, 6, 8),
                (AV, 1, 6), ("fin", 1, 0, 4),
                (S, 3, 0), (S, 2, 6),
                (AV, 2, 0), (AV, 2, 2), (AV, 2, 4),
                (S, 3, 2), (S, 2, 8),
                ("v", 3),
                (AV, 2, 6),
                (S, 3, 4), (S, 2, 10),
                (AV, 2, 8), (AV, 2, 10), ("fin", 2, 0, 4),
                (S, 3, 6), (S, 3, 8),
                (AV, 3, 0), (AV, 3, 2),
                (S, 3, 10), (S, 3, 12),
                (AV, 3, 4), (AV, 3, 6), (AV, 3, 8),
                (S, 3, 14),
                (AV, 3, 10), (AV, 3, 12), ("fin", 3, 0, 2),
                (AV, 3, 14), ("fin", 3, 2, 4),
            ]
            pts = {}
            for act in actions:
                if act[0] == "qk":
                    proj_qk(act[1], act[2], act[3])
                elif act[0] == "v":
                    proj_v(act[1])
                elif act[0] == S:
                    pts[act[1:]] = s_unit(act[1:])
                elif act[0] == AV:
                    av_unit(act[1:], pts.pop(act[1:]))
                else:
                    finalize(act[1], act[2], act[3])
            assert not pts

    nc.finalize()
    return nc


_NC_CACHE = None
LAST_RESULTS = None


def _pack(w, cols):
    # [C, cols] -> [128, CT, cols] with partition p holding rows {ct*128+p}
    return np.ascontiguousarray(
        np.asarray(w, np.float32).reshape(CT, 128, cols).transpose(1, 0, 2)
    ).astype(ml_dtypes.bfloat16)


def kernel(x, Wq, Wk, Wv, trace=False, **run_kwargs):
    global _NC_CACHE, LAST_RESULTS
    x = np.asarray(x, dtype=np.float32)
    # k in the psum low half, q in the high half (see build_nc)
    wqk = _pack(np.concatenate([np.asarray(Wk, np.float32),
                                np.asarray(Wq, np.float32)], axis=1), 128)
    wv = _pack(Wv, D)

    if _NC_CACHE is None:
        _NC_CACHE = build_nc()
    nc = _NC_CACHE

    in_maps = []
    for b in range(N_CORES):
        # xT[p, ct, t] = x[b, t, ct*128+p]
        xT = np.ascontiguousarray(
            x[b].T.reshape(CT, 128, T).transpose(1, 0, 2)
        ).astype(ml_dtypes.bfloat16)
        in_maps.append({"xT": xT, "wqk": wqk, "wv": wv})

    res = run_bass_kernel_spmd(
        nc, in_maps, core_ids=list(range(N_CORES)), trace=trace, **run_kwargs
    )
    LAST_RESULTS = res
    out = np.empty((N_CORES, T, D), dtype=np.float32)
    for b in range(N_CORES):
        y = res.results[b]["y"]  # [128, NT, D]
        out[b] = np.asarray(y, dtype=np.float32).transpose(1, 0, 2).reshape(T, D)
    return out


if __name__ == "__main__":
    rng = np.random.default_rng(0)
    x = rng.standard_normal((B, T, C), dtype=np.float32)
    s = 1.0 / np.sqrt(C)
    Wq = rng.standard_normal((C, D), dtype=np.float32) * s
    Wk = rng.standard_normal((C, D), dtype=np.float32) * s
    Wv = rng.standard_normal((C, D), dtype=np.float32) * s
    out = kernel(x, Wq, Wk, Wv)
    print("out", out.shape, out.dtype, float(np.abs(out).max()))
